# revision 1
# baseline (speedup 1.0000x reference)
"""IntraAttention Trainium2 kernel, 8-core SPMD.

Reference computation (N=4096 rows, d=1024):
    Q = X @ Wq.T + bq ; K = X @ Wk.T + bk ; V = X @ Wv.T + bv
    alpha = softmax(Q @ K.T / sqrt(d), axis=1)
    V_ = alpha @ V
    x = concat([V_, Q], axis=1)              # [N, 2d]
    x1 = x @ Wl.T + bl                        # [N, d]
    h = x @ Wa.T + ba                         # [N, 2d]
    out = x1 * (h[:, :d] * sigmoid(h[:, d:]))

Sharding: rows of X are sharded across 8 cores (512 rows each). Q stays
local; K and V shards are all-gathered (K as [d, rows] blocks, V as
[rows, d] blocks) in two pipelined chunks each, so each core runs its
512xN attention + GLU chain locally while the gathers fly. The Q-only
halves of the x1/h projections run while the first gather completes.

On-chip everything is computed transposed ([feature, row] layout) so all
matmul contractions run along the partition axis with N=512 moving free
dim. Matmul operands are fp16 (fp22 multiply, fp32 accumulate in PSUM);
biases/normalization/final multiply run in fp32.
"""

import numpy as np

import concourse.bass as bass
import concourse.bacc as bacc
import concourse.tile as tile
import concourse.bass_utils as bass_utils
from concourse import mybir

P = 128            # partitions
D = 1024           # model dim
N = 4096           # rows
NCORES = 8
R = N // NCORES    # rows per core = 512
HR = R // 2        # half of the local rows = 256
DC = D // P        # d chunks = 8
NK = N // P        # key tiles = 32
TD = 2 * D         # 2048
TDC = TD // P      # 16
HK = 4             # k-chunks of the g0 h-projection computed early (Q part)

F32 = mybir.dt.float32
F16 = mybir.dt.float16

RG = [list(range(NCORES))]

# key-tile visit order: (ss-major) so the first 16 tiles only need chunk 0
# of the K/V allgathers. kt_global = rr*4 + ss; softmax/attention are
# permutation-invariant over keys as long as exp tile i pairs with V rows
# of the same key tile.
KT_ORDER = [(rr, ss) for ss in range(4) for rr in range(NCORES)]


def build_nc():
    nc = bacc.Bacc(
        "TRN2",
        target_bir_lowering=False,
        debug=False,
        num_devices=NCORES,
    )

    # ---- per-core I/O ----
    xt = nc.dram_tensor("xt", [D, R], F16, kind="ExternalInput")      # X_c.T
    wqt = nc.dram_tensor("wqt", [D, D], F16, kind="ExternalInput")    # Wq.T
    wkt = nc.dram_tensor("wkt", [D, D], F16, kind="ExternalInput")    # Wk.T
    wvt = nc.dram_tensor("wvt", [D, D], F16, kind="ExternalInput")    # Wv.T
    wlt = nc.dram_tensor("wlt", [TD, D], F16, kind="ExternalInput")   # Wl.T
    wat = nc.dram_tensor("wat", [TD, TD], F16, kind="ExternalInput")  # Wa.T
    bq = nc.dram_tensor("bq", [P, DC], F32, kind="ExternalInput")
    bk = nc.dram_tensor("bk", [P, DC], F32, kind="ExternalInput")
    bvb = nc.dram_tensor("bvb", [P, D], F32, kind="ExternalInput")    # bv bcast
    bl = nc.dram_tensor("bl", [P, DC], F32, kind="ExternalInput")
    ba = nc.dram_tensor("ba", [P, TDC], F32, kind="ExternalInput")
    out = nc.dram_tensor("out", [D, R], F32, kind="ExternalOutput")   # out_c.T

    # ---- collective buffers ----
    ktc_d = [nc.dram_tensor(f"ktc_d{h}", [D, HR], F16) for h in range(2)]
    vc_d = [nc.dram_tensor(f"vc_d{h}", [HR, D], F16) for h in range(2)]
    ag_k = [nc.dram_tensor(f"ag_k{h}", [NCORES * D, HR], F16, addr_space="Shared")
            for h in range(2)]
    ag_v = [nc.dram_tensor(f"ag_v{h}", [NCORES * HR, D], F16, addr_space="Shared")
            for h in range(2)]

    with tile.TileContext(nc) as tc:
        with (
            tc.tile_pool(name="cpool", bufs=1) as cpool,
            tc.tile_pool(name="wpool", bufs=10) as wpool,
            tc.tile_pool(name="vlpool", bufs=4) as vlpool,
            tc.tile_pool(name="pspool", bufs=8, space="PSUM") as pspool,
        ):
            # constants (scalar-engine DMA queue; sync queue kept for bulk)
            bq_t = cpool.tile([P, DC], F32, name="bq_t")
            bk_t = cpool.tile([P, DC], F32, name="bk_t")
            bl_t = cpool.tile([P, DC], F32, name="bl_t")
            ba_t = cpool.tile([P, TDC], F32, name="ba_t")
            bvb_t = cpool.tile([P, D], F32, name="bvb_t")
            nc.scalar.dma_start(bq_t, bq[:, :])
            nc.scalar.dma_start(bk_t, bk[:, :])
            nc.scalar.dma_start(bl_t, bl[:, :])
            nc.scalar.dma_start(ba_t, ba[:, :])
            nc.scalar.dma_start(bvb_t, bvb[:, :])
            ones_t = cpool.tile([P, 1], F16, name="ones_t")
            nc.vector.memset(ones_t, 1.0)
            ones_row = cpool.tile([1, P], F32, name="ones_row")
            nc.vector.memset(ones_row, 1.0)

            with tc.tile_pool(name="qpool", bufs=1) as qpool, \
                 tc.tile_pool(name="vtpool", bufs=1) as vtpool, \
                 tc.tile_pool(name="qfpool", bufs=1) as qfpool:

                # ============ QKV projections + allgathers ============
                with tc.tile_pool(name="xpool", bufs=1) as xpool, \
                     tc.tile_pool(name="stpool", bufs=4) as stpool:
                    xt_t = [xpool.tile([P, R], F16, name=f"xt{k}") for k in range(DC)]

                    # --- K_c.T = Wk @ X_c.T + bk ---  (first: feeds AG(K))
                    kt_ps = [pspool.tile([P, R], F32, name=f"ktps{m}", tag="ps")
                             for m in range(DC)]
                    for k in range(DC):
                        nc.sync.dma_start(xt_t[k], xt[k * P:(k + 1) * P, :])
                        wk_t = wpool.tile([P, D], F16, name="wk_t", tag="w")
                        nc.sync.dma_start(wk_t, wkt[k * P:(k + 1) * P, :])
                        for m in range(DC):
                            nc.tensor.matmul(
                                kt_ps[m], wk_t[:, m * P:(m + 1) * P], xt_t[k],
                                start=(k == 0), stop=(k == DC - 1))
                    for m in range(DC):
                        st = stpool.tile([P, R], F16, name="st_k", tag="st")
                        nc.vector.tensor_scalar_add(st, kt_ps[m], bk_t[:, m:m + 1])
                        for h in range(2):
                            nc.scalar.dma_start(
                                ktc_d[h][m * P:(m + 1) * P, :],
                                st[:, h * HR:(h + 1) * HR])
                    for h in range(2):
                        nc.gpsimd.collective_compute(
                            "AllGather", mybir.AluOpType.bypass, replica_groups=RG,
                            ins=[ktc_d[h].ap().opt()], outs=[ag_k[h].ap().opt()])

                    # --- V_c = X_c @ Wv.T + bv ---
                    v_ps = [pspool.tile([P, R], F32, name=f"vps{i}", tag="ps")
                            for i in range(8)]
                    for k in range(DC):
                        wv_t = wpool.tile([P, D], F16, name="wv_t", tag="w")
                        nc.sync.dma_start(wv_t, wvt[k * P:(k + 1) * P, :])
                        for rt in range(4):
                            for db in range(2):
                                nc.tensor.matmul(
                                    v_ps[rt * 2 + db],
                                    xt_t[k][:, rt * P:(rt + 1) * P],
                                    wv_t[:, db * 512:(db + 1) * 512],
                                    start=(k == 0), stop=(k == DC - 1))
                    for rt in range(4):
                        for db in range(2):
                            st = stpool.tile([P, R], F16, name="st_v", tag="st")
                            nc.vector.tensor_add(
                                st, v_ps[rt * 2 + db], bvb_t[:, db * 512:(db + 1) * 512])
                            nc.scalar.dma_start(
                                vc_d[rt // 2][(rt % 2) * P:(rt % 2 + 1) * P,
                                              db * 512:(db + 1) * 512], st)
                    for h in range(2):
                        nc.gpsimd.collective_compute(
                            "AllGather", mybir.AluOpType.bypass, replica_groups=RG,
                            ins=[vc_d[h].ap().opt()], outs=[ag_v[h].ap().opt()])

                    # --- Q_c.T = Wq @ X_c.T + bq ---
                    qt_t = [qpool.tile([P, R], F16, name=f"qt{m}") for m in range(DC)]
                    q_ps = [pspool.tile([P, R], F32, name=f"qps{m}", tag="ps")
                            for m in range(DC)]
                    for k in range(DC):
                        wq_t = wpool.tile([P, D], F16, name="wq_t", tag="w")
                        nc.sync.dma_start(wq_t, wqt[k * P:(k + 1) * P, :])
                        for m in range(DC):
                            nc.tensor.matmul(
                                q_ps[m], wq_t[:, m * P:(m + 1) * P], xt_t[k],
                                start=(k == 0), stop=(k == DC - 1))
                    for m in range(DC):
                        nc.vector.tensor_scalar_add(qt_t[m], q_ps[m], bq_t[:, m:m + 1])

                # ---- gap fillers while AG(K0) completes ----
                # x1_q.T[m] = Wl[:, D:].T-chunks @ Q.T (+ bl folded in)
                x1q_t = [qfpool.tile([P, R], F32, name=f"x1q{m}") for m in range(DC)]
                x1q_ps = [pspool.tile([P, R], F32, name=f"x1qps{m}", tag="ps")
                          for m in range(DC)]
                for k in range(DC):
                    wl_t = wpool.tile([P, D], F16, name="wl_t", tag="w")
                    nc.sync.dma_start(wl_t, wlt[(DC + k) * P:(DC + k + 1) * P, :])
                    for m in range(DC):
                        nc.tensor.matmul(
                            x1q_ps[m], wl_t[:, m * P:(m + 1) * P], qt_t[k],
                            start=(k == 0), stop=(k == DC - 1))
                for m in range(DC):
                    nc.vector.tensor_scalar_add(x1q_t[m], x1q_ps[m], bl_t[:, m:m + 1])

                # tail HK k-chunks of h-g0's Q part (+ ba folded in)
                hq0_t = [qfpool.tile([P, R], F32, name=f"hq0_{m}") for m in range(DC)]
                hq0_ps = [pspool.tile([P, R], F32, name=f"hq0ps{m}", tag="ps")
                          for m in range(DC)]
                for j in range(HK):
                    k = TDC - HK + j
                    wa_t = wpool.tile([P, D], F16, name="wa_t", tag="w")
                    nc.sync.dma_start(wa_t, wat[k * P:(k + 1) * P, 0:D])
                    for m in range(DC):
                        nc.tensor.matmul(
                            hq0_ps[m], wa_t[:, m * P:(m + 1) * P], qt_t[k - DC],
                            start=(j == 0), stop=(j == HK - 1))
                for m in range(DC):
                    nc.vector.tensor_scalar_add(hq0_t[m], hq0_ps[m], ba_t[:, m:m + 1])

                # ============ scoresT + exp + sums ============
                with tc.tile_pool(name="epool", bufs=1) as epool:
                    exp_t = [epool.tile([P, R], F16, name=f"exp{i}")
                             for i in range(NK)]
                    sums_ps = pspool.tile([1, R], F32, name="sums_ps", tag="ps")

                    def sums_mm(i):
                        nc.tensor.matmul(
                            sums_ps, ones_t, exp_t[i],
                            start=(i == 0), stop=(i == NK - 1),
                            skip_group_check=True)

                    for i, (rr, ss) in enumerate(KT_ORDER):
                        h, sh = (0, ss) if ss < 2 else (1, ss - 2)
                        kl = wpool.tile([P, D], F16, name="kl", tag="w")
                        eng = nc.sync if i % 2 == 0 else nc.scalar
                        eng.dma_start(
                            kl.rearrange("p (c n) -> p c n", c=DC),
                            ag_k[h][rr * D:(rr + 1) * D, sh * P:(sh + 1) * P]
                            .rearrange("(c p) n -> p c n", p=P))
                        sc_ps = pspool.tile([P, R], F32, name="sc_ps", tag="ps")
                        for c in range(DC):
                            nc.tensor.matmul(
                                sc_ps, kl[:, c * P:(c + 1) * P], qt_t[c],
                                start=(c == 0), stop=(c == DC - 1))
                        nc.scalar.activation(
                            exp_t[i], sc_ps, mybir.ActivationFunctionType.Exp,
                            bias=0.0, scale=1.0 / 32.0)
                        if i > 0:
                            sums_mm(i - 1)    # one behind: exp(i-1) surely done
                    sums_mm(NK - 1)

                    # reciprocal + broadcast to all partitions
                    recip_t = cpool.tile([1, R], F32, name="recip_t")
                    nc.vector.reciprocal(recip_t, sums_ps)
                    bc_ps = pspool.tile([P, R], F32, name="bc_ps", tag="ps")
                    nc.tensor.matmul(bc_ps, ones_row, recip_t, start=True, stop=True)
                    bc_t = cpool.tile([P, R], F32, name="bc_t")
                    nc.vector.tensor_copy(bc_t, bc_ps)

                    # ============ V_T = (alpha @ V).T ============
                    vt_ps = [pspool.tile([P, R], F32, name=f"vtps{m}", tag="ps")
                             for m in range(DC)]
                    for i, (rr, ss) in enumerate(KT_ORDER):
                        h, sh = (0, ss) if ss < 2 else (1, ss - 2)
                        vl = vlpool.tile([P, D], F16, name="vl", tag="vl")
                        eng = nc.sync if i % 2 == 0 else nc.scalar
                        eng.dma_start(
                            vl, ag_v[h][rr * HR + sh * P:rr * HR + (sh + 1) * P, :])
                        for m in range(DC):
                            nc.tensor.matmul(
                                vt_ps[m], vl[:, m * P:(m + 1) * P], exp_t[i],
                                start=(i == 0), stop=(i == NK - 1),
                                skip_group_check=True)
                    vt_t = [vtpool.tile([P, R], F16, name=f"vt{m}")
                            for m in range(DC)]
                    for m in range(DC):
                        nc.vector.tensor_mul(vt_t[m], vt_ps[m], bc_t)

                # ============ x1 (V-half), h, GLU ============
                def xk(k):
                    return vt_t[k] if k < DC else qt_t[k - DC]

                with tc.tile_pool(name="fpool", bufs=1) as fpool, \
                     tc.tile_pool(name="wg1pool", bufs=1) as wg1pool:
                    x1_ps = [pspool.tile([P, R], F32, name=f"x1ps{m}", tag="ps")
                             for m in range(DC)]
                    for k in range(DC):
                        wl_t = wpool.tile([P, D], F16, name="wl_t", tag="w")
                        nc.sync.dma_start(wl_t, wlt[k * P:(k + 1) * P, :])
                        for m in range(DC):
                            nc.tensor.matmul(
                                x1_ps[m], wl_t[:, m * P:(m + 1) * P], vt_t[k],
                                start=(k == 0), stop=(k == DC - 1))
                    x1_t = [fpool.tile([P, R], F32, name=f"x1{m}") for m in range(DC)]
                    for m in range(DC):
                        nc.vector.tensor_add(x1_t[m], x1_ps[m], x1q_t[m])

                    # h group 0 (a part): k-chunks [0, TDC-HK), Q-tail was
                    # precomputed into hq0_t
                    a_t = [fpool.tile([P, R], F32, name=f"a{m}") for m in range(DC)]
                    h_ps = [pspool.tile([P, R], F32, name=f"hps0_{m}", tag="ps")
                            for m in range(DC)]
                    for k in range(TDC - HK):
                        wa_t = wpool.tile([P, D], F16, name="wa_t", tag="w")
                        nc.sync.dma_start(wa_t, wat[k * P:(k + 1) * P, 0:D])
                        for m in range(DC):
                            nc.tensor.matmul(
                                h_ps[m], wa_t[:, m * P:(m + 1) * P], xk(k),
                                start=(k == 0), stop=(k == TDC - HK - 1))
                    for m in range(DC):
                        nc.vector.tensor_add(a_t[m], h_ps[m], hq0_t[m])

                    # h group 1 (b part): preload all 16 wa tiles, loop
                    # m-outer so each output column block finishes early and
                    # the GLU/output tail overlaps remaining matmuls.
                    wg1_t = [wg1pool.tile([P, D], F16, name=f"wg1_{k}")
                             for k in range(TDC)]
                    for k in range(TDC):
                        eng = nc.sync if k % 2 == 0 else nc.scalar
                        eng.dma_start(wg1_t[k], wat[k * P:(k + 1) * P, D:TD])
                    for m in range(DC):
                        hg1 = pspool.tile([P, R], F32, name=f"hps1_{m}", tag="ps")
                        for k in range(TDC):
                            nc.tensor.matmul(
                                hg1, wg1_t[k][:, m * P:(m + 1) * P], xk(k),
                                start=(k == 0), stop=(k == TDC - 1))
                        sig = fpool.tile([P, R], F32, name="sig", tag="sig", bufs=2)
                        nc.scalar.activation(
                            sig, hg1, mybir.ActivationFunctionType.Sigmoid,
                            bias=ba_t[:, DC + m:DC + m + 1], scale=1.0)
                        nc.vector.tensor_mul(a_t[m], a_t[m], sig)
                        nc.vector.tensor_mul(a_t[m], x1_t[m], a_t[m])
                        nc.scalar.dma_start(out[m * P:(m + 1) * P, :], a_t[m])

    nc.compile()
    return nc


_NC = None


def _get_nc():
    global _NC
    if _NC is None:
        _NC = build_nc()
    return _NC


def make_in_maps(input_features, Wq, bq, Wk, bk, Wv, bv, Wl, bl, Wa, ba):
    f = np.ascontiguousarray
    x = np.asarray(input_features, dtype=np.float32)
    xt_full = f(x.T.astype(np.float16))                  # [D, N]
    wqt = f(np.asarray(Wq, np.float32).T.astype(np.float16))
    wkt = f(np.asarray(Wk, np.float32).T.astype(np.float16))
    wvt = f(np.asarray(Wv, np.float32).T.astype(np.float16))
    wlt = f(np.asarray(Wl, np.float32).T.astype(np.float16))   # [2D, D]
    wat = f(np.asarray(Wa, np.float32).T.astype(np.float16))   # [2D, 2D]
    bq_r = f(np.asarray(bq, np.float32).reshape(DC, P).T)      # [P, DC]
    bk_r = f(np.asarray(bk, np.float32).reshape(DC, P).T)
    bl_r = f(np.asarray(bl, np.float32).reshape(DC, P).T)
    ba_r = f(np.asarray(ba, np.float32).reshape(TDC, P).T)     # [P, TDC]
    bvb = f(np.broadcast_to(np.asarray(bv, np.float32), (P, D)))
    in_maps = []
    for c in range(NCORES):
        in_maps.append({
            "xt": f(xt_full[:, c * R:(c + 1) * R]),
            "wqt": wqt, "wkt": wkt, "wvt": wvt, "wlt": wlt, "wat": wat,
            "bq": bq_r, "bk": bk_r, "bvb": bvb, "bl": bl_r, "ba": ba_r,
        })
    return in_maps


def run(in_maps, trace=False):
    nc = _get_nc()
    return bass_utils.run_bass_kernel_spmd(
        nc, in_maps, core_ids=list(range(NCORES)), trace=trace)


def kernel(input_features, Wq, bq, Wk, bk, Wv, bv, Wl, bl, Wa, ba):
    in_maps = make_in_maps(input_features, Wq, bq, Wk, bk, Wv, bv, Wl, bl, Wa, ba)
    res = run(in_maps)
    out = np.empty((N, D), dtype=np.float32)
    for c in range(NCORES):
        out[c * R:(c + 1) * R, :] = res.results[c]["out"].T
    return out



# revision 7
# speedup vs baseline: 1.4206x; 1.4206x over previous
"""IntraAttention Trainium2 kernel, 8-core SPMD, mixed fp8/fp16.

Reference computation (N=4096 rows, d=1024):
    Q = X @ Wq.T + bq ; K = X @ Wk.T + bk ; V = X @ Wv.T + bv
    alpha = softmax(Q @ K.T / sqrt(d), axis=1)
    V_ = alpha @ V
    x = concat([V_, Q], axis=1)              # [N, 2d]
    x1 = x @ Wl.T + bl                        # [N, d]
    h = x @ Wa.T + ba                         # [N, 2d]
    out = x1 * (h[:, :d] * sigmoid(h[:, d:]))

Sharding: rows of X are sharded across 8 cores (512 rows each). Q stays
local; K and V shards are all-gathered in fp8 (two pipelined chunks
each) while the Q-side fp16 projections (Q, and the Q-halves of
x1/h_a/h_b) run as gap fillers.

Precision: softmax averaging makes the whole attention path noise
immune, so K/V projections, scores, exp, and alpha@V run as fp8e4
DoubleRow matmuls (2x PE throughput, 256-deep contraction per
instruction). V_ is tiny relative to Q, so the V-halves of the
x1/h projections are fp8 DoubleRow too. Only the Q projection and the
Q-halves of x1/h_a/h_b stay fp16 (they dominate output accuracy).
All matmuls accumulate fp32 in PSUM.

All DMAs move >=1KB contiguous per partition: weights and X are
pre-arranged on the host into [P, chunk, ...] layouts, and the K
allgather buffer is packed per-partition so score K-tiles load as
[128, 1KB] lines.
"""

import numpy as np
import ml_dtypes

import concourse.bass as bass
import concourse.bacc as bacc
import concourse.tile as tile
import concourse.bass_utils as bass_utils
from concourse import mybir

P = 128            # partitions
D = 1024           # model dim
N = 4096           # rows
NCORES = 8
R = N // NCORES    # rows per core = 512
HR = R // 2        # half of the local rows = 256
DC = D // P        # d chunks = 8
DP = DC // 2       # d chunk pairs = 4
NK = N // P        # key tiles = 32
NKP = NK // 2      # key tile pairs = 16
TD = 2 * D         # 2048
TDC = TD // P      # 16

F32 = mybir.dt.float32
F16 = mybir.dt.float16
F8 = mybir.dt.float8e4
DRM = mybir.MatmulPerfMode.DoubleRow
F8NP = ml_dtypes.float8_e4m3

RG = [list(range(NCORES))]

# key-tile visit order (ss-major): the first 16 tiles only need chunk 0
# of the K/V allgathers. kt pairs (i=2t, 2t+1) share the same ss (and
# hence the same allgather chunk); exp tile slot pairing must match the
# V row pairing in alpha@V.
KT_ORDER = [(rr, ss) for ss in range(4) for rr in range(NCORES)]


def build_nc():
    nc = bacc.Bacc(
        "TRN2",
        target_bir_lowering=False,
        debug=False,
        num_devices=NCORES,
    )

    # ---- per-core I/O (host pre-arranged layouts, see make_in_maps) ----
    xt8 = nc.dram_tensor("xt8", [P, DP * 2 * R], F8, kind="ExternalInput")
    xt16 = nc.dram_tensor("xt16", [P, DC * R], F16, kind="ExternalInput")
    wq16 = nc.dram_tensor("wq16", [P, DC * D], F16, kind="ExternalInput")
    wk8 = nc.dram_tensor("wk8", [P, DP * 2 * D], F8, kind="ExternalInput")
    wv8 = nc.dram_tensor("wv8", [P, DP * 2 * D], F8, kind="ExternalInput")
    wlq16 = nc.dram_tensor("wlq16", [P, DC * D], F16, kind="ExternalInput")
    waq16 = nc.dram_tensor("waq16", [P, DC * D], F16, kind="ExternalInput")
    wbq16 = nc.dram_tensor("wbq16", [P, DC * D], F16, kind="ExternalInput")
    wlv8 = nc.dram_tensor("wlv8", [P, DP * 2 * D], F8, kind="ExternalInput")
    wav8 = nc.dram_tensor("wav8", [P, DP * 2 * D], F8, kind="ExternalInput")
    wbv8 = nc.dram_tensor("wbv8", [P, DP * 2 * D], F8, kind="ExternalInput")
    bq = nc.dram_tensor("bq", [P, DC], F32, kind="ExternalInput")
    bk = nc.dram_tensor("bk", [P, DC], F32, kind="ExternalInput")
    bvb = nc.dram_tensor("bvb", [P, D], F32, kind="ExternalInput")    # bv bcast
    bl = nc.dram_tensor("bl", [P, DC], F32, kind="ExternalInput")
    ba = nc.dram_tensor("ba", [P, TDC], F32, kind="ExternalInput")
    out = nc.dram_tensor("out", [D, R], F32, kind="ExternalOutput")   # out_c.T

    # ---- collective buffers (fp8) ----
    # K half h: [p, sh, m, n] = K.T[d = m*128+p, key = h*256 + sh*128 + n]
    ktc_d = [nc.dram_tensor(f"ktc_d{h}", [P, 2 * DC * P], F8) for h in range(2)]
    vc_d = [nc.dram_tensor(f"vc_d{h}", [HR, D], F8) for h in range(2)]
    ag_k = [nc.dram_tensor(f"ag_k{h}", [NCORES * P, 2 * DC * P], F8,
                           addr_space="Shared") for h in range(2)]
    ag_v = [nc.dram_tensor(f"ag_v{h}", [NCORES * HR, D], F8,
                           addr_space="Shared") for h in range(2)]

    with tile.TileContext(nc) as tc:
        with (
            tc.tile_pool(name="cpool", bufs=1) as cpool,
            tc.tile_pool(name="wpool", bufs=10) as wpool,
            tc.tile_pool(name="pspool", bufs=8, space="PSUM") as pspool,
        ):
            # constants on the scalar DMA queue
            bq_t = cpool.tile([P, DC], F32, name="bq_t")
            bk_t = cpool.tile([P, DC], F32, name="bk_t")
            bl_t = cpool.tile([P, DC], F32, name="bl_t")
            ba_t = cpool.tile([P, TDC], F32, name="ba_t")
            bvb_t = cpool.tile([P, D], F32, name="bvb_t")
            nc.scalar.dma_start(bq_t, bq[:, :])
            nc.scalar.dma_start(bk_t, bk[:, :])
            nc.scalar.dma_start(bl_t, bl[:, :])
            nc.scalar.dma_start(ba_t, ba[:, :])
            nc.scalar.dma_start(bvb_t, bvb[:, :])
            # DoubleRow pair-dim stride must be a multiple of 16 elements,
            # so the ones column is padded to 16.
            ones8 = cpool.tile([P, 2, 16], F8, name="ones8")
            nc.vector.memset(ones8, 1.0)
            ones_row = cpool.tile([1, P], F32, name="ones_row")
            nc.vector.memset(ones_row, 1.0)

            with tc.tile_pool(name="qpool", bufs=1) as qpool, \
                 tc.tile_pool(name="qfpool", bufs=1) as qfpool, \
                 tc.tile_pool(name="gwpool", bufs=1) as gwpool:

                # ============ K/V projections (fp8 DR) + allgathers ============
                with tc.tile_pool(name="xpool", bufs=1) as xpool, \
                     tc.tile_pool(name="kvpool", bufs=1) as kvpool:
                    x8_t = [xpool.tile([P, 2, R], F8, name=f"x8_{kp}")
                            for kp in range(DP)]

                    # --- K_c.T = Wk @ X_c.T + bk ---  (first: feeds AG(K))
                    kt_ps = [pspool.tile([P, R], F32, name=f"ktps{m}", tag="ps")
                             for m in range(DC)]
                    for kp in range(DP):
                        nc.sync.dma_start(
                            x8_t[kp],
                            xt8[:, kp * 2 * R:(kp + 1) * 2 * R]
                            .rearrange("p (j n) -> p j n", j=2))
                        wk_t = wpool.tile([P, 2, D], F8, name="wk_t", tag="w")
                        nc.scalar.dma_start(
                            wk_t,
                            wk8[:, kp * 2 * D:(kp + 1) * 2 * D]
                            .rearrange("p (j m) -> p j m", j=2))
                        for m in range(DC):
                            nc.tensor.matmul(
                                kt_ps[m], wk_t[:, :, m * P:(m + 1) * P], x8_t[kp],
                                start=(kp == 0), stop=(kp == DP - 1),
                                perf_mode=DRM)
                    # pack into per-partition AG layout and ship
                    k8h = [kvpool.tile([P, 2 * DC * P], F8, name=f"k8h{h}")
                           for h in range(2)]
                    for m in range(DC):
                        for h in range(2):
                            nc.vector.tensor_scalar_add(
                                k8h[h].rearrange("p (s m n) -> p s m n",
                                                 s=2, m=DC)[:, :, m, :],
                                kt_ps[m][:, h * HR:(h + 1) * HR]
                                .rearrange("p (s n) -> p s n", s=2),
                                bk_t[:, m:m + 1])
                    for h in range(2):
                        nc.scalar.dma_start(ktc_d[h][:, :], k8h[h])
                        nc.gpsimd.collective_compute(
                            "AllGather", mybir.AluOpType.bypass, replica_groups=RG,
                            ins=[ktc_d[h].ap().opt()], outs=[ag_k[h].ap().opt()])

                    # --- V_c = X_c @ Wv.T + bv ---
                    v_ps = [pspool.tile([P, R], F32, name=f"vps{i}", tag="ps")
                            for i in range(8)]
                    for kp in range(DP):
                        wv_t = wpool.tile([P, 2, D], F8, name="wv_t", tag="w")
                        nc.scalar.dma_start(
                            wv_t,
                            wv8[:, kp * 2 * D:(kp + 1) * 2 * D]
                            .rearrange("p (j m) -> p j m", j=2))
                        for rt in range(4):
                            for db in range(2):
                                nc.tensor.matmul(
                                    v_ps[rt * 2 + db],
                                    x8_t[kp][:, :, rt * P:(rt + 1) * P],
                                    wv_t[:, :, db * 512:(db + 1) * 512],
                                    start=(kp == 0), stop=(kp == DP - 1),
                                    perf_mode=DRM)
                    v8h = [kvpool.tile([P, 2 * D], F8, name=f"v8h{h}")
                           for h in range(2)]
                    for rt in range(4):
                        for db in range(2):
                            nc.vector.tensor_add(
                                v8h[rt // 2][:, (rt % 2) * D + db * 512:
                                             (rt % 2) * D + (db + 1) * 512],
                                v_ps[rt * 2 + db],
                                bvb_t[:, db * 512:(db + 1) * 512])
                    for h in range(2):
                        nc.scalar.dma_start(
                            vc_d[h].rearrange("(x p) f -> p x f", p=P),
                            v8h[h].rearrange("p (x f) -> p x f", x=2))
                        nc.gpsimd.collective_compute(
                            "AllGather", mybir.AluOpType.bypass, replica_groups=RG,
                            ins=[vc_d[h].ap().opt()], outs=[ag_v[h].ap().opt()])

                    # --- Q_c.T = Wq @ X_c.T + bq --- (fp16)
                    xt_t = [xpool.tile([P, R], F16, name=f"xt{k}")
                            for k in range(DC)]
                    qt16 = [qpool.tile([P, R], F16, name=f"qt{m}")
                            for m in range(DC)]
                    q8p = [qpool.tile([P, 2, R], F8, name=f"q8p{mp}")
                           for mp in range(DP)]
                    q_ps = [pspool.tile([P, R], F32, name=f"qps{m}", tag="ps")
                            for m in range(DC)]
                    for k in range(DC):
                        nc.sync.dma_start(xt_t[k], xt16[:, k * R:(k + 1) * R])
                        wq_t = wpool.tile([P, D], F16, name="wq_t", tag="w")
                        nc.sync.dma_start(wq_t, wq16[:, k * D:(k + 1) * D])
                        for m in range(DC):
                            nc.tensor.matmul(
                                q_ps[m], wq_t[:, m * P:(m + 1) * P], xt_t[k],
                                start=(k == 0), stop=(k == DC - 1))
                    for m in range(DC):
                        nc.vector.tensor_scalar_add(
                            qt16[m], q_ps[m], bq_t[:, m:m + 1])
                        nc.vector.tensor_scalar_add(
                            q8p[m // 2][:, m % 2, :], q_ps[m], bq_t[:, m:m + 1])

                # ---- gap fillers while AG(K)/AG(V) complete ----
                # Q-halves of x1 / h_a / h_b in fp16, biases folded in,
                # partials staged to SBUF fp16.
                fill_spec = [
                    ("x1q", wlq16, bl_t, 0),
                    ("haq", waq16, ba_t, 0),
                    ("hbq", wbq16, ba_t, DC),
                ]
                fills = {}
                for fname, wsrc, bias_t, bcol in fill_spec:
                    f_t = [qfpool.tile([P, R], F16, name=f"{fname}_{m}")
                           for m in range(DC)]
                    f_ps = [pspool.tile([P, R], F32, name=f"{fname}ps{m}",
                                        tag="ps") for m in range(DC)]
                    for k in range(DC):
                        wf_t = wpool.tile([P, D], F16, name=f"w_{fname}", tag="w")
                        nc.sync.dma_start(wf_t, wsrc[:, k * D:(k + 1) * D])
                        for m in range(DC):
                            nc.tensor.matmul(
                                f_ps[m], wf_t[:, m * P:(m + 1) * P], qt16[k],
                                start=(k == 0), stop=(k == DC - 1))
                    for m in range(DC):
                        nc.vector.tensor_scalar_add(
                            f_t[m], f_ps[m], bias_t[:, bcol + m:bcol + m + 1])
                    fills[fname] = f_t

                # GLU fp8 weight preloads (vector DMA queue, idle here)
                wlv_t = [gwpool.tile([P, 2, D], F8, name=f"wlv{mp}")
                         for mp in range(DP)]
                wav_t = [gwpool.tile([P, 2, D], F8, name=f"wav{mp}")
                         for mp in range(DP)]
                wbv_t = [gwpool.tile([P, 2, D], F8, name=f"wbv{mp}")
                         for mp in range(DP)]
                for mp in range(DP):
                    for wt, wsrc in ((wlv_t, wlv8), (wav_t, wav8), (wbv_t, wbv8)):
                        nc.gpsimd.dma_start(
                            wt[mp],
                            wsrc[:, mp * 2 * D:(mp + 1) * 2 * D]
                            .rearrange("p (j m) -> p j m", j=2))

                # ============ scoresT + exp + sums (fp8 DR) ============
                with tc.tile_pool(name="epool", bufs=1) as epool, \
                     tc.tile_pool(name="klpool", bufs=4) as klpool, \
                     tc.tile_pool(name="vlpool", bufs=1) as vlpool, \
                     tc.tile_pool(name="vtpool", bufs=1) as vtpool:
                    exp8 = [epool.tile([P, 2, R], F8, name=f"exp{t}")
                            for t in range(NKP)]
                    sums_ps = pspool.tile([1, R], F32, name="sums_ps", tag="ps")

                    def sums_mm(t):
                        nc.tensor.matmul(
                            sums_ps, ones8[:, :, 0:1], exp8[t],
                            start=(t == 0), stop=(t == NKP - 1),
                            perf_mode=DRM, skip_group_check=True)

                    for i, (rr, ss) in enumerate(KT_ORDER):
                        h, sh = divmod(ss, 2)
                        kl8 = klpool.tile([P, DC, P], F8, name="kl8", tag="kl")
                        eng = nc.sync if i % 2 == 0 else nc.gpsimd
                        eng.dma_start(
                            kl8,
                            ag_k[h][rr * P:(rr + 1) * P,
                                    sh * DC * P:(sh + 1) * DC * P]
                            .rearrange("p (c n) -> p c n", n=P))
                        sc_ps = pspool.tile([P, R], F32, name="sc_ps", tag="ps")
                        for mp in range(DP):
                            nc.tensor.matmul(
                                sc_ps, kl8[:, 2 * mp:2 * mp + 2, :], q8p[mp],
                                start=(mp == 0), stop=(mp == DP - 1),
                                perf_mode=DRM)
                        nc.scalar.activation(
                            exp8[i // 2][:, i % 2, :], sc_ps,
                            mybir.ActivationFunctionType.Exp,
                            bias=0.0, scale=1.0 / 32.0)
                        if i % 2 == 1 and i >= 3:
                            sums_mm(i // 2 - 1)   # one pair behind

                    # ============ V_T = (alpha @ V).T, two m-passes ============
                    vl8 = [vlpool.tile([P, 2, D], F8, name=f"vl8_{t}")
                           for t in range(NKP)]
                    vt_ps = [pspool.tile([P, R], F32, name=f"vtps{m}", tag="ps")
                             for m in range(4)]
                    recip_t = cpool.tile([1, R], F32, name="recip_t")
                    bc_t = cpool.tile([P, R], F32, name="bc_t")
                    for t in range(NKP):
                        for j in range(2):
                            rr, ss = KT_ORDER[2 * t + j]
                            h, sh = divmod(ss, 2)
                            eng = nc.sync if j == 0 else nc.gpsimd
                            eng.dma_start(
                                vl8[t][:, j, :],
                                ag_v[h][rr * HR + sh * P:rr * HR + (sh + 1) * P, :])
                        for m in range(4):
                            nc.tensor.matmul(
                                vt_ps[m], vl8[t][:, :, m * P:(m + 1) * P], exp8[t],
                                start=(t == 0), stop=(t == NKP - 1),
                                perf_mode=DRM, skip_group_check=True)
                        if t == 0:
                            sums_mm(NKP - 1)
                            nc.vector.reciprocal(recip_t, sums_ps)
                        if t == 8:
                            bc_ps = pspool.tile([P, R], F32, name="bc_ps",
                                                tag="ps")
                            nc.tensor.matmul(bc_ps, ones_row, recip_t,
                                             start=True, stop=True,
                                             skip_group_check=True)
                            nc.vector.tensor_copy(bc_t, bc_ps)

                    vt8 = [vtpool.tile([P, 2, R], F8, name=f"vt8_{mp}")
                           for mp in range(DP)]
                    vt_ps2 = [pspool.tile([P, R], F32, name=f"vtps2_{m}",
                                          tag="ps") for m in range(4)]
                    for t in range(NKP):
                        for m in range(4):
                            nc.tensor.matmul(
                                vt_ps2[m], vl8[t][:, :, (m + 4) * P:(m + 5) * P],
                                exp8[t],
                                start=(t == 0), stop=(t == NKP - 1),
                                perf_mode=DRM, skip_group_check=True)
                        if t == 0:
                            for m in range(4):   # normalize first half
                                nc.vector.tensor_mul(
                                    vt8[m // 2][:, m % 2, :], vt_ps[m], bc_t)
                    for m in range(4, DC):
                        nc.vector.tensor_mul(
                            vt8[m // 2][:, m % 2, :], vt_ps2[m - 4], bc_t)

                    # ============ V-halves of x1/h (fp8 DR) + GLU ============
                    with tc.tile_pool(name="fpool", bufs=1) as fpool:
                        for m in range(DC):
                            x1v_ps = pspool.tile([P, R], F32, name="x1v_ps",
                                                 tag="ps")
                            hav_ps = pspool.tile([P, R], F32, name="hav_ps",
                                                 tag="ps")
                            hbv_ps = pspool.tile([P, R], F32, name="hbv_ps",
                                                 tag="ps")
                            for mp in range(DP):
                                nc.tensor.matmul(
                                    x1v_ps, wlv_t[mp][:, :, m * P:(m + 1) * P],
                                    vt8[mp], start=(mp == 0), stop=(mp == DP - 1),
                                    perf_mode=DRM)
                            for mp in range(DP):
                                nc.tensor.matmul(
                                    hav_ps, wav_t[mp][:, :, m * P:(m + 1) * P],
                                    vt8[mp], start=(mp == 0), stop=(mp == DP - 1),
                                    perf_mode=DRM)
                            for mp in range(DP):
                                nc.tensor.matmul(
                                    hbv_ps, wbv_t[mp][:, :, m * P:(m + 1) * P],
                                    vt8[mp], start=(mp == 0), stop=(mp == DP - 1),
                                    perf_mode=DRM)
                            bt = fpool.tile([P, R], F32, name="bt", tag="bt",
                                            bufs=2)
                            nc.vector.tensor_add(bt, hbv_ps, fills["hbq"][m])
                            sig = fpool.tile([P, R], F32, name="sig", tag="sig",
                                             bufs=2)
                            nc.scalar.activation(
                                sig, bt, mybir.ActivationFunctionType.Sigmoid,
                                bias=0.0, scale=1.0)
                            x1_t = fpool.tile([P, R], F32, name="x1_t", tag="x1",
                                              bufs=2)
                            nc.vector.tensor_add(x1_t, x1v_ps, fills["x1q"][m])
                            a_t = fpool.tile([P, R], F32, name="a_t", tag="at",
                                             bufs=2)
                            nc.vector.tensor_add(a_t, hav_ps, fills["haq"][m])
                            nc.vector.tensor_mul(a_t, a_t, sig)
                            nc.vector.tensor_mul(a_t, x1_t, a_t)
                            nc.scalar.dma_start(out[m * P:(m + 1) * P, :], a_t)

    nc.compile()
    return nc


_NC = None


def _get_nc():
    global _NC
    if _NC is None:
        _NC = build_nc()
    return _NC


def _pair_layout(wT, dtype):
    """[K, C] (rows=contraction) -> [P, (K//256)*2*C] DoubleRow layout."""
    K, C = wT.shape
    a = wT.reshape(K // 256, 2, P, C).transpose(2, 0, 1, 3).reshape(P, -1)
    return np.ascontiguousarray(a.astype(dtype))


def _chunk_layout(wT, dtype):
    """[K, C] (rows=contraction) -> [P, (K//128)*C] chunk layout."""
    K, C = wT.shape
    a = wT.reshape(K // P, P, C).transpose(1, 0, 2).reshape(P, -1)
    return np.ascontiguousarray(a.astype(dtype))


def make_in_maps(input_features, Wq, bq, Wk, bk, Wv, bv, Wl, bl, Wa, ba):
    f = np.ascontiguousarray
    x = np.asarray(input_features, dtype=np.float32)
    xt_full = x.T                                          # [D, N] fp32
    wqT = np.asarray(Wq, np.float32).T
    wkT = np.asarray(Wk, np.float32).T
    wvT = np.asarray(Wv, np.float32).T
    wlT = np.asarray(Wl, np.float32).T                     # [2D, D]
    waT = np.asarray(Wa, np.float32).T                     # [2D, 2D]

    wq16 = _chunk_layout(wqT, np.float16)
    wk8 = _pair_layout(wkT, F8NP)
    wv8 = _pair_layout(wvT, F8NP)
    wlq16 = _chunk_layout(wlT[D:], np.float16)
    waq16 = _chunk_layout(waT[D:, :D], np.float16)
    wbq16 = _chunk_layout(waT[D:, D:], np.float16)
    wlv8 = _pair_layout(wlT[:D], F8NP)
    wav8 = _pair_layout(waT[:D, :D], F8NP)
    wbv8 = _pair_layout(waT[:D, D:], F8NP)

    bq_r = f(np.asarray(bq, np.float32).reshape(DC, P).T)  # [P, DC]
    bk_r = f(np.asarray(bk, np.float32).reshape(DC, P).T)
    bl_r = f(np.asarray(bl, np.float32).reshape(DC, P).T)
    ba_r = f(np.asarray(ba, np.float32).reshape(TDC, P).T)  # [P, TDC]
    bvb = f(np.broadcast_to(np.asarray(bv, np.float32), (P, D)))

    in_maps = []
    for c in range(NCORES):
        xt_c = xt_full[:, c * R:(c + 1) * R]               # [D, R]
        in_maps.append({
            "xt8": _pair_layout(xt_c, F8NP),
            "xt16": _chunk_layout(xt_c, np.float16),
            "wq16": wq16, "wk8": wk8, "wv8": wv8,
            "wlq16": wlq16, "waq16": waq16, "wbq16": wbq16,
            "wlv8": wlv8, "wav8": wav8, "wbv8": wbv8,
            "bq": bq_r, "bk": bk_r, "bvb": bvb, "bl": bl_r, "ba": ba_r,
        })
    return in_maps


def run(in_maps, trace=False):
    nc = _get_nc()
    return bass_utils.run_bass_kernel_spmd(
        nc, in_maps, core_ids=list(range(NCORES)), trace=trace)


def kernel(input_features, Wq, bq, Wk, bk, Wv, bv, Wl, bl, Wa, ba):
    in_maps = make_in_maps(input_features, Wq, bq, Wk, bk, Wv, bv, Wl, bl, Wa, ba)
    res = run(in_maps)
    out = np.empty((N, D), dtype=np.float32)
    for c in range(NCORES):
        out[c * R:(c + 1) * R, :] = res.results[c]["out"].T
    return out
